# revision 8
# baseline (speedup 1.0000x reference)
"""Two-layer DGL-style GraphConv (norm='both') on 8 TRN2 NeuronCores.

Strategy (self-contained; shapes hardcoded for the 100k-node / 3.2M-edge problem):
 - Host: compute degree norms, pre-scale x by norm_src, sort edges by
   (dst-core, dst-tile-of-128, src-chunk-of-25088), pad each group to 128-edge
   blocks (padding shared across cores so the SPMD program is identical).
 - Device, phase 1 (per core): for each of 98 dst tiles, dma_gather the edge
   source rows (int16 local indices into one of 4 row-chunks of the padded
   node table), build one-hot selector matrices with a stride-0 iota compare
   on DVE, segment-sum via PE matmuls accumulating in PSUM (transposed
   orientation aggT[feat,dst]), then zT = W0^T @ aggT,
   h1sT = relu(zT*nd*ns + b0*ns), H1' = h1sT^T @ W1 -> local [12544,64].
 - XLA-level all_gather of the slices -> full H1' [100352,64] per core.
 - Device, phase 2: same gather/segment-sum on H1' (same indices), then
   out = agg2*nd + b1 -> local [12544,64] slice; host concatenates.
"""
import numpy as np

N_CORES = 8
P = 128
NBLK_MAX_SPLIT = 1024  # max idxs per dma_gather call: 64 descriptors x 16 engines

_cache = {}

L1_INS = ["xs", "gidx", "dls", "ndns", "b0ns", "iota", "W0", "W1"]
L2_INS = ["gidx", "dls", "ndc", "b1r", "iota"]


def _preprocess(x, edge_index, W0, b0, W1, b1):
    N, IN_F = x.shape
    HID = W0.shape[1]
    OUT_F = W1.shape[1]
    E = edge_index.shape[1]
    PER = N // N_CORES                      # 12500 dsts per core
    T = (PER + P - 1) // P                  # 98 tiles per core
    PERP = T * P                            # 12544 padded
    NPAD = PERP * N_CORES                   # 100352 padded rows
    NCH = 4
    CHROWS = NPAD // NCH                    # 25088 rows per gather chunk

    src = edge_index[0].astype(np.int64)
    dst = edge_index[1].astype(np.int64)

    deg_out = np.bincount(src, minlength=N).astype(np.float32)
    deg_in = np.bincount(dst, minlength=N).astype(np.float32)
    ns_all = 1.0 / np.sqrt(np.maximum(deg_out, 1.0))
    nd_all = 1.0 / np.sqrt(np.maximum(deg_in, 1.0))

    xs = x.astype(np.float32) * ns_all[:, None]
    xs_pad = np.zeros((NPAD, IN_F), np.float32)
    for c in range(N_CORES):
        xs_pad[c * PERP:c * PERP + PER] = xs[c * PER:(c + 1) * PER]

    srcrow = (src // PER) * PERP + (src % PER)   # row in padded layout
    dstc = dst // PER
    dstj = dst % PER
    tt = dstj // P
    dl = (dstj % P).astype(np.float32)
    ch = srcrow // CHROWS
    lid = (srcrow % CHROWS).astype(np.int32)

    key = ((dstc * T + tt) * NCH + ch).astype(np.int64)
    order = np.argsort(key, kind="stable")
    s_lid = lid[order]
    s_dl = dl[order]
    counts = np.bincount(key, minlength=N_CORES * T * NCH).reshape(N_CORES, T, NCH)
    starts = np.zeros(N_CORES * T * NCH + 1, np.int64)
    np.cumsum(counts.reshape(-1), out=starts[1:])

    cmax = counts.max(axis=0)                       # [T, NCH]
    nblk_tc = (cmax + P - 1) // P                   # [T, NCH] blocks

    calls = []        # [T] -> list of (chunk, nblk, blk_off); shared by cores
    blk_off = 0
    for t in range(T):
        tl = []
        for c2 in range(NCH):
            nb = int(nblk_tc[t, c2])
            while nb > 0:
                take = min(nb, NBLK_MAX_SPLIT // P)
                tl.append((c2, take, blk_off))
                blk_off += take
                nb -= take
        calls.append(tl)
    TB = blk_off

    gidx_cores, dl_cores = [], []
    for c in range(N_CORES):
        idx_flat = np.zeros(TB * P, np.int32)
        dl_flat = np.full(TB * P, -1.0, np.float32)
        for t in range(T):
            covered = {}
            for (c2, nb, boff) in calls[t]:
                covered.setdefault(c2, []).append((nb, boff))
            for c2, pieces in covered.items():
                g = (c * T + t) * NCH + c2
                k = int(counts[c, t, c2])
                st = int(starts[g])
                li = s_lid[st:st + k]
                di = s_dl[st:st + k]
                pos = 0
                for (nb, boff) in pieces:
                    cap = nb * P
                    take = min(cap, k - pos)
                    if take > 0:
                        idx_flat[boff * P:boff * P + take] = li[pos:pos + take]
                        dl_flat[boff * P:boff * P + take] = di[pos:pos + take]
                    pos += take
        dl_slab = dl_flat.reshape(TB, P).T.copy()
        gw = np.zeros((P, TB * 8), np.int16)
        for t in range(T):
            for (c2, nb, boff) in calls[t]:
                ni = nb * P
                arr = idx_flat[boff * P:boff * P + ni]
                w = arr.reshape(ni // 16, 16).T.astype(np.int16)
                gw[:, boff * 8:boff * 8 + ni // 16] = np.tile(w, (8, 1))
        gidx_cores.append(gw)
        dl_cores.append(dl_slab)

    ndns_rep, b0ns_rep, ndc_col = [], [], []
    for c in range(N_CORES):
        ns_c = np.ones(PERP, np.float32)
        nd_c = np.zeros(PERP, np.float32)
        ns_c[:PER] = ns_all[c * PER:(c + 1) * PER]
        nd_c[:PER] = nd_all[c * PER:(c + 1) * PER]
        ndns = (ns_c * nd_c).reshape(T, P)
        ndns_rep.append(np.broadcast_to(ndns[:, None, :], (T, P, P)).astype(np.float32).copy())
        b0ns = b0.astype(np.float32)[None, :, None] * ns_c.reshape(T, 1, P)
        b0ns_rep.append(np.broadcast_to(b0ns, (T, P, P)).astype(np.float32).copy())
        ndc_col.append(nd_c.reshape(T, P).T.copy())

    iota = np.tile(np.arange(P, dtype=np.float32), (P, 1))
    b1r = np.tile(b1.astype(np.float32), (P, 1))

    struct = dict(N=N, E=E, IN_F=IN_F, HID=HID, OUT_F=OUT_F, PER=PER, T=T,
                  PERP=PERP, NPAD=NPAD, NCH=NCH, CHROWS=CHROWS, TB=TB,
                  calls=calls)
    in_maps = []
    for c in range(N_CORES):
        in_maps.append({
            "xs": xs_pad,
            "gidx": gidx_cores[c],
            "dls": dl_cores[c],
            "ndns": ndns_rep[c],
            "b0ns": b0ns_rep[c],
            "ndc": ndc_col[c],
            "b1r": b1r,
            "iota": iota,
            "W0": W0.astype(np.float32),
            "W1": W1.astype(np.float32),
        })
    return struct, in_maps


def _seg_sum_tile(nc, mybir, tl, gidx_t, dls_t, iota_t, src_dram, feat,
                  CHROWS, wp, acc_psum, next_q, swap):
    """Gather + selector-build + segment matmuls for one dst tile.
    swap=False: out[dst, feat] (lhsT=S);  swap=True: out[feat, dst] (lhsT=g)."""
    f32 = mybir.dt.float32
    first = True
    for ci, (c2, nb, boff) in enumerate(tl):
        ni = nb * P
        g = wp.tile([P, nb, feat], f32, tag=f"g{feat}")
        nc.gpsimd.dma_gather(
            g[:], src_dram[c2 * CHROWS:(c2 + 1) * CHROWS, :],
            gidx_t[:, boff * 8:boff * 8 + ni // 16],
            ni, ni, feat, queue_num=next_q())
        S = wp.tile([P, nb * P], f32, tag="S1")
        nc.vector.tensor_tensor(
            out=S[:].rearrange("p (b f) -> p b f", b=nb),
            in0=iota_t[:, None, :].to_broadcast([P, nb, P]),
            in1=dls_t[:, boff:boff + nb, None].to_broadcast([P, nb, P]),
            op=mybir.AluOpType.is_equal)
        for b in range(nb):
            stop = (ci == len(tl) - 1) and (b == nb - 1)
            if swap:
                nc.tensor.matmul(acc_psum[:], lhsT=g[:, b, :],
                                 rhs=S[:, b * P:(b + 1) * P], start=first, stop=stop)
            else:
                nc.tensor.matmul(acc_psum[:], lhsT=S[:, b * P:(b + 1) * P],
                                 rhs=g[:, b, :], start=first, stop=stop)
            first = False


def _build_l1(struct):
    from concourse import bacc, mybir, tile

    T = struct["T"]; TB = struct["TB"]; calls = struct["calls"]
    IN_F = struct["IN_F"]; HID = struct["HID"]; OUT_F = struct["OUT_F"]
    PERP = struct["PERP"]; NPAD = struct["NPAD"]; CHROWS = struct["CHROWS"]
    f32 = mybir.dt.float32

    nc = bacc.Bacc("TRN2", target_bir_lowering=False, num_devices=N_CORES,
                   num_swdge_queues=4, dynamic_dma_scratch_size=65536)
    xs = nc.dram_tensor("xs", [NPAD, IN_F], f32, kind="ExternalInput")
    gidx = nc.dram_tensor("gidx", [P, TB * 8], mybir.dt.int16, kind="ExternalInput")
    dls = nc.dram_tensor("dls", [P, TB], f32, kind="ExternalInput")
    ndns = nc.dram_tensor("ndns", [T, P, P], f32, kind="ExternalInput")
    b0ns = nc.dram_tensor("b0ns", [T, P, P], f32, kind="ExternalInput")
    iota = nc.dram_tensor("iota", [P, P], f32, kind="ExternalInput")
    W0 = nc.dram_tensor("W0", [IN_F, HID], f32, kind="ExternalInput")
    W1 = nc.dram_tensor("W1", [HID, OUT_F], f32, kind="ExternalInput")
    h1s_out = nc.dram_tensor("h1s_out", [PERP, OUT_F], f32, kind="ExternalOutput")

    qn = [0]

    def next_q():
        q = qn[0] % 4
        qn[0] += 1
        return q

    with tile.TileContext(nc) as tc:
        with tc.tile_pool(name="const", bufs=1) as cp, \
             tc.tile_pool(name="work", bufs=3) as wp, \
             tc.tile_pool(name="sc", bufs=3) as scp, \
             tc.tile_pool(name="ev", bufs=3) as ep, \
             tc.tile_pool(name="psA", bufs=2, space="PSUM") as pA, \
             tc.tile_pool(name="psB", bufs=2, space="PSUM") as pB:

            gidx_t = cp.tile([P, TB * 8], mybir.dt.int16)
            nc.sync.dma_start(out=gidx_t[:], in_=gidx[:])
            dls_t = cp.tile([P, TB], f32)
            nc.sync.dma_start(out=dls_t[:], in_=dls[:])
            iota_t = cp.tile([P, P], f32)
            nc.sync.dma_start(out=iota_t[:], in_=iota[:])
            W0_t = cp.tile([IN_F, HID], f32)
            nc.sync.dma_start(out=W0_t[:], in_=W0[:])
            W1_t = cp.tile([HID, OUT_F], f32)
            nc.sync.dma_start(out=W1_t[:], in_=W1[:])

            for t in range(T):
                aggT = pA.tile([P, P], f32, tag="aggT")
                _seg_sum_tile(nc, mybir, calls[t], gidx_t, dls_t, iota_t, xs,
                              IN_F, CHROWS, wp, aggT, next_q, swap=True)
                aggT_sb = ep.tile([P, P], f32, tag="aggT_sb")
                nc.vector.tensor_copy(aggT_sb[:], aggT[:])
                zT = pB.tile([P, P], f32, tag="zT")
                nc.tensor.matmul(zT[:], lhsT=W0_t[:], rhs=aggT_sb[:], start=True, stop=True)
                ndns_t = scp.tile([P, P], f32, tag="ndns")
                nc.sync.dma_start(out=ndns_t[:], in_=ndns[t, :, :])
                b0ns_t = scp.tile([P, P], f32, tag="b0ns")
                nc.sync.dma_start(out=b0ns_t[:], in_=b0ns[t, :, :])
                u1 = ep.tile([P, P], f32, tag="u1")
                nc.vector.tensor_tensor(out=u1[:], in0=zT[:], in1=ndns_t[:],
                                        op=mybir.AluOpType.mult)
                u2 = ep.tile([P, P], f32, tag="u2")
                nc.vector.tensor_tensor(out=u2[:], in0=u1[:], in1=b0ns_t[:],
                                        op=mybir.AluOpType.add)
                h1sT = ep.tile([P, P], f32, tag="h1sT")
                nc.scalar.activation(h1sT[:], u2[:], mybir.ActivationFunctionType.Relu)
                h1o = pB.tile([P, OUT_F], f32, tag="h1o")
                nc.tensor.matmul(h1o[:], lhsT=h1sT[:], rhs=W1_t[:], start=True, stop=True)
                h1p = ep.tile([P, OUT_F], f32, tag="h1p")
                nc.vector.tensor_copy(h1p[:], h1o[:])
                nc.sync.dma_start(out=h1s_out[t * P:(t + 1) * P, :], in_=h1p[:])

    nc.compile()
    return nc


def _build_l2(struct):
    from concourse import bacc, mybir, tile

    T = struct["T"]; TB = struct["TB"]; calls = struct["calls"]
    OUT_F = struct["OUT_F"]
    PERP = struct["PERP"]; NPAD = struct["NPAD"]; CHROWS = struct["CHROWS"]
    f32 = mybir.dt.float32

    nc = bacc.Bacc("TRN2", target_bir_lowering=False, num_devices=N_CORES,
                   num_swdge_queues=4, dynamic_dma_scratch_size=65536)
    h1full = nc.dram_tensor("h1full", [NPAD, OUT_F], f32, kind="ExternalInput")
    gidx = nc.dram_tensor("gidx", [P, TB * 8], mybir.dt.int16, kind="ExternalInput")
    dls = nc.dram_tensor("dls", [P, TB], f32, kind="ExternalInput")
    ndc = nc.dram_tensor("ndc", [P, T], f32, kind="ExternalInput")
    b1r = nc.dram_tensor("b1r", [P, OUT_F], f32, kind="ExternalInput")
    iota = nc.dram_tensor("iota", [P, P], f32, kind="ExternalInput")
    outp = nc.dram_tensor("outp", [PERP, OUT_F], f32, kind="ExternalOutput")

    qn = [0]

    def next_q():
        q = qn[0] % 4
        qn[0] += 1
        return q

    with tile.TileContext(nc) as tc:
        with tc.tile_pool(name="const", bufs=1) as cp, \
             tc.tile_pool(name="work", bufs=3) as wp, \
             tc.tile_pool(name="ev", bufs=3) as ep, \
             tc.tile_pool(name="psA", bufs=2, space="PSUM") as pA:

            gidx_t = cp.tile([P, TB * 8], mybir.dt.int16)
            nc.sync.dma_start(out=gidx_t[:], in_=gidx[:])
            dls_t = cp.tile([P, TB], f32)
            nc.sync.dma_start(out=dls_t[:], in_=dls[:])
            iota_t = cp.tile([P, P], f32)
            nc.sync.dma_start(out=iota_t[:], in_=iota[:])
            ndc_t = cp.tile([P, T], f32)
            nc.sync.dma_start(out=ndc_t[:], in_=ndc[:])
            b1r_t = cp.tile([P, OUT_F], f32)
            nc.sync.dma_start(out=b1r_t[:], in_=b1r[:])

            for t in range(T):
                agg2 = pA.tile([P, OUT_F], f32, tag="agg2")
                _seg_sum_tile(nc, mybir, calls[t], gidx_t, dls_t, iota_t, h1full,
                              OUT_F, CHROWS, wp, agg2, next_q, swap=False)
                o1 = ep.tile([P, OUT_F], f32, tag="o1")
                nc.vector.tensor_scalar(out=o1[:], in0=agg2[:],
                                        scalar1=ndc_t[:, t:t + 1], scalar2=None,
                                        op0=mybir.AluOpType.mult)
                o2 = ep.tile([P, OUT_F], f32, tag="o2")
                nc.vector.tensor_tensor(out=o2[:], in0=o1[:], in1=b1r_t[:],
                                        op=mybir.AluOpType.add)
                nc.sync.dma_start(out=outp[t * P:(t + 1) * P, :], in_=o2[:])

    nc.compile()
    return nc


def _exec_info(nc):
    from concourse import mybir
    in_names, out_names, out_avals = [], [], []
    pid = nc.partition_id_tensor.name if nc.partition_id_tensor else None
    for alloc in nc.m.functions[0].allocations:
        if not isinstance(alloc, mybir.MemoryLocationSet):
            continue
        name = alloc.memorylocations[0].name
        if alloc.kind == "ExternalInput":
            if name != pid:
                in_names.append(name)
        elif alloc.kind == "ExternalOutput":
            out_names.append(name)
            out_avals.append((tuple(alloc.tensor_shape), mybir.dt.np(alloc.dtype)))
    return in_names, out_names, out_avals, pid


def _make_runner(nc1, nc2, struct):
    import jax
    from jax.sharding import Mesh, PartitionSpec
    from jax.experimental.shard_map import shard_map
    from concourse import bass2jax

    bass2jax.install_neuronx_cc_hook()

    in1, out1, av1, pid1 = _exec_info(nc1)
    in2, out2, av2, pid2 = _exec_info(nc2)
    assert out1 == ["h1s_out"] and out2 == ["outp"], (out1, out2)

    devices = jax.devices()[:N_CORES]
    mesh = Mesh(np.asarray(devices), ("core",))

    def _make_phase(nc, in_names, out_names, out_avals, pid):
        n_in = len(in_names)
        n_out = len(out_avals)

        def body(*args):
            operands = list(args)
            all_names = list(in_names) + list(out_names)
            if pid:
                operands.append(bass2jax.partition_id_tensor())
                all_names.append(pid)
            outs = bass2jax._bass_exec_p.bind(
                *operands,
                out_avals=tuple(jax.core.ShapedArray(s, d) for (s, d) in out_avals),
                in_names=tuple(all_names),
                out_names=tuple(out_names),
                lowering_input_output_aliases=(),
                sim_require_finite=True,
                sim_require_nnan=True,
                nc=nc)
            return tuple(outs)

        donate = tuple(range(n_in, n_in + n_out))
        return jax.jit(shard_map(body, mesh=mesh,
                                 in_specs=(PartitionSpec("core"),) * (n_in + n_out),
                                 out_specs=(PartitionSpec("core"),) * n_out,
                                 check_rep=False),
                       donate_argnums=donate, keep_unused=True)

    fn1 = _make_phase(nc1, in1, out1, av1, pid1)
    fn2 = _make_phase(nc2, in2, out2, av2, pid2)

    def ag_body(x):
        return jax.lax.all_gather(x, "core", axis=0, tiled=True)

    fn_ag = jax.jit(shard_map(ag_body, mesh=mesh,
                              in_specs=(PartitionSpec("core"),),
                              out_specs=PartitionSpec("core"), check_rep=False))

    def run(named_global_args):
        a1 = [named_global_args[nm] for nm in in1]
        z1 = [np.zeros((N_CORES * s[0],) + tuple(s[1:]), d) for (s, d) in av1]
        (h1slice,) = fn1(*a1, *z1)
        h1full = fn_ag(h1slice)
        m2 = dict(named_global_args)
        m2["h1full"] = h1full
        a2 = [m2[nm] for nm in in2]
        z2 = [np.zeros((N_CORES * s[0],) + tuple(s[1:]), d) for (s, d) in av2]
        (outp,) = fn2(*a2, *z2)
        return outp

    return run


def _concat_args(in_maps):
    named = {}
    for nm in L1_INS + ["ndc", "b1r"]:
        named[nm] = np.concatenate([in_maps[c][nm] for c in range(N_CORES)], axis=0)
    return named


def kernel(**inputs):
    x = np.asarray(inputs["x"])
    edge_index = np.asarray(inputs["edge_index"])
    W0 = np.asarray(inputs["W0"]); b0 = np.asarray(inputs["b0"])
    W1 = np.asarray(inputs["W1"]); b1 = np.asarray(inputs["b1"])

    struct, in_maps = _preprocess(x, edge_index, W0, b0, W1, b1)
    ckey = (x.shape, edge_index.shape, W0.shape, W1.shape, struct["TB"],
            tuple(tuple(tl) for tl in struct["calls"]))
    if ckey not in _cache:
        _cache.clear()
        nc1 = _build_l1(struct)
        nc2 = _build_l2(struct)
        fn = _make_runner(nc1, nc2, struct)
        _cache[ckey] = (nc1, nc2, fn)
    nc1, nc2, fn = _cache[ckey]

    import jax
    out_all = np.asarray(jax.block_until_ready(fn(_concat_args(in_maps))))
    PERP, PER = struct["PERP"], struct["PER"]
    out_all = out_all.reshape(N_CORES, PERP, struct["OUT_F"])
    out = np.concatenate([out_all[c][:PER] for c in range(N_CORES)], axis=0)
    if np.issubdtype(x.dtype, np.floating):
        out = out.astype(x.dtype)
    return out
